# revision 48
# baseline (speedup 1.0000x reference)
"""Causal multi-head attention with RoPE for Trainium2, 8-core SPMD.

Problem: B=2, S=2048, D_MODEL=1024, H=16, HD=64, causal softmax(QK^T/8)V
with interleaved-pair RoPE on q/k, projections Wq/Wk/Wv/Wo.

Sharding (host side): batch x head-group. Core c handles batch b=c//4 and
head group g=c%4 (heads 4g..4g+3, a 256-wide slice of the projection dims).
Each core computes a full [S, D_MODEL] partial of the output (its head
group's contribution through Wo) in bf16; host casts to f32 and sums 4
partials per batch.

Device strategy (all matmuls bf16, fp32 accumulate):
 - emission interleaves projection m-tiles, attention q-chunks and o_proj
   tiles at score-group granularity so the PE stream always has work while
   ACT chews through the exp backlog; input DMAs are chunked so the first
   projection starts ~4.5us in
 - host permutes Wq/Wk rows per head to [evens(32) | odds(32)] so RoPE
   reads contiguous blocks: two full-width muls (qkf*cos16, qkf*sin16) +
   strided-block add/sub on DVE; scores are invariant to the permutation
   since q and k share it
 - Q,K projected in [s, o] layout -> RoPE -> one batched DMA transpose per
   m-tile into qkt4 [128, 4, S]; QK projection PSUM double-buffered across
   two tags, V accumulates via the PV psum ring
 - scoresT[k, q] = Kt.T @ Qt per 128-key block, head pairs row-packed on PE
   partitions 0:64/64:128; wide [128, 1024] PSUM score tiles, one Exp per
   (group, head) on ACT writing probs into per-(hp,head) SBUF buffers
   (qc0-2 share one set; qc3 gets its own carved from the released
   phase-1 pool so exp(qc3) never waits on PV(qc2)); causal diagonal
   masked by Pool multiply
 - PV flipped: out[q, h, hd] with lhsT = probs block [keys, q], rhs =
   [V | 1] [keys, 65] -- N=65 per matmul (the cost driver is the moving
   dim) instead of 128-512; col 64 accumulates the softmax denominator per
   q partition, so normalization is one reciprocal + per-partition
   tensor_scalar on Pool
 - y [q, hd] normalized then batch-transposed into yt2 [128, 2, S];
   o_proj per q-chunk with po PSUM rotating over freed phase-1/score
   banks, evacuation split DVE/ACT at the tail, out DMAs in bf16
   alternating between the SP and ACT DMA queues
"""

import numpy as np
import ml_dtypes

B, S, D, H = 2, 2048, 1024, 16
HD = 64
NCORES = 8
HEADS_PER_CORE = 4
GDIM = HEADS_PER_CORE * HD          # 256 projection cols per core
SB = S // 128                        # 16 s-tiles
KD = D // 128                        # 8 k-tiles over d
QCHUNK = 512
NQC = S // QCHUNK                    # 4 q-chunks
WIDE = 1024                          # wide scores psum tile (2 banks)

_BF16 = ml_dtypes.bfloat16
_cache = {}


def _score_layout(qc):
    """Per (qc): list of (kb, qoff, n) in emission order and the global column
    base of each kb block in the pe probs buffer; plus chunking into <=WIDE
    score-psum groups. Returns (groups, base) where groups is a list of
    [(kb, qoff, n, colbase), ...] and base maps kb -> global pe column."""
    q0 = qc * QCHUNK
    order = list(range(4 * qc)) + [4 * qc, 4 * qc + 1, 4 * qc + 3, 4 * qc + 2]
    base = {}
    blocks = []
    pos = 0
    for kb in order:
        r = max(0, kb - 4 * qc)
        qoff = q0 + r * 128 if kb >= 4 * qc else q0
        n = QCHUNK - r * 128 if kb >= 4 * qc else QCHUNK
        base[kb] = pos
        blocks.append((kb, qoff, n, pos))
        pos += n
    groups, cur, cols = [], [], 0
    for (kb, qoff, n, colbase) in blocks:
        if cols + n > WIDE:
            groups.append(cur)
            cur, cols = [], 0
        cur.append((kb, qoff, n, colbase))
        cols += n
    groups.append(cur)
    return groups, base, pos


def _build(use_rope: bool):
    import concourse.bass as bass
    import concourse.mybir as mybir
    import concourse.tile as tile
    from concourse import bacc
    from contextlib import ExitStack

    F32 = mybir.dt.float32
    BF16 = mybir.dt.bfloat16
    EXP = mybir.ActivationFunctionType.Exp
    MULT = mybir.AluOpType.mult

    nc = bacc.Bacc(None, target_bir_lowering=False)

    xt_d = nc.dram_tensor("xt", [D, S], BF16, kind="ExternalInput")
    wqk_d = nc.dram_tensor("wqk", [D, 2 * GDIM], BF16, kind="ExternalInput")
    wv_d = nc.dram_tensor("wv", [D, GDIM], BF16, kind="ExternalInput")
    wo_d = nc.dram_tensor("wo", [GDIM, D], BF16, kind="ExternalInput")
    cos_d = nc.dram_tensor("cos8", [S, 256], BF16, kind="ExternalInput")
    sin_d = nc.dram_tensor("sin8", [S, 256], BF16, kind="ExternalInput")
    mask_d = nc.dram_tensor("maskT", [128, 128], BF16, kind="ExternalInput")
    out_d = nc.dram_tensor("out", [S, D], BF16, kind="ExternalOutput")

    # pe probs buffer column count for the widest chunk (qc=3)
    _, _, NCOLS = _score_layout(NQC - 1)

    xt_dr = xt_d.rearrange("(k p) s -> p k s", p=128)
    wqk_dr = wqk_d.rearrange("(k p) o -> p k o", p=128)
    cos_dr = cos_d.rearrange("(m p) f -> p m f", p=128)
    sin_dr = sin_d.rearrange("(m p) f -> p m f", p=128)

    with tile.TileContext(nc) as tc:
        es = ExitStack()
        big = es.enter_context(tc.tile_pool(name="big", bufs=1))
        work = es.enter_context(tc.tile_pool(name="work", bufs=2))
        scp = es.enter_context(tc.tile_pool(name="sc", bufs=1, space="PSUM"))
        yqp = es.enter_context(tc.tile_pool(name="yq", bufs=2, space="PSUM"))

        # ---- resident tiles ----
        wo = big.tile([128, 2, D], BF16)
        maskT = big.tile([128, 128], BF16)
        qkt4 = big.tile([128, 4, S], BF16)
        vsb = big.tile([128, SB, HEADS_PER_CORE * 65], BF16)
        yt2 = big.tile([128, 2, S], BF16)
        # probs buffers for qc0-2 (max 5376 cols); qc3 gets its own buffers
        # carved out of the released phase-1 pool so exp(qc3) need not wait
        # for PV(qc2) to drain these
        _, _, NC2 = _score_layout(2)
        _, _, NC1 = _score_layout(1)
        pe_main = [[big.tile([128, NC2], BF16, tag=f"pe{hp}{i}",
                             name=f"pe{hp}{i}") for i in range(2)]
                   for hp in range(2)]
        # phase-1-only tensors: released after the last projection m-tile
        ph1_ctx = tc.tile_pool(name="ph1", bufs=1)
        ph1 = ph1_ctx.__enter__()
        xt = ph1.tile([128, KD, S], BF16)
        wqk = ph1.tile([128, KD, 2 * GDIM], BF16)
        wv = ph1.tile([128, KD, GDIM], BF16)
        if use_rope:
            cos8 = ph1.tile([128, SB, 256], BF16)
            sin8 = ph1.tile([128, SB, 256], BF16)

        # ones columns of [V | 1] (memset before anything else)
        vsb4 = vsb.rearrange("p m (h c) -> p m h c", h=4)
        nc.vector.memset(vsb4[:, :, :, 64:65], 1.0)

        # ---- input DMAs, chunked so m-tile 0 unblocks early; weights go
        # down the ACT queue in parallel with xt on the SP queue ----
        nc.sync.dma_start(wqk[:, 0:2, :], wqk_dr[:, 0:2, :])
        nc.sync.dma_start(xt[:, 0:2, 0:QCHUNK], xt_dr[:, 0:2, 0:QCHUNK])
        nc.sync.dma_start(wqk[:, 2:4, :], wqk_dr[:, 2:4, :])
        nc.sync.dma_start(xt[:, 2:4, 0:QCHUNK], xt_dr[:, 2:4, 0:QCHUNK])
        nc.sync.dma_start(wqk[:, 4:6, :], wqk_dr[:, 4:6, :])
        nc.sync.dma_start(xt[:, 4:6, 0:QCHUNK], xt_dr[:, 4:6, 0:QCHUNK])
        nc.sync.dma_start(wqk[:, 6:8, :], wqk_dr[:, 6:8, :])
        nc.sync.dma_start(xt[:, 6:8, 0:QCHUNK], xt_dr[:, 6:8, 0:QCHUNK])
        nc.sync.dma_start(wv[:], wv_d.rearrange("(k p) o -> p k o", p=128))
        if use_rope:
            nc.sync.dma_start(cos8[:, 0:4, :], cos_dr[:, 0:4, :])
            nc.sync.dma_start(sin8[:, 0:4, :], sin_dr[:, 0:4, :])
        nc.sync.dma_start(maskT[:], mask_d[:])
        for c in range(1, 4):
            cs = slice(c * QCHUNK, (c + 1) * QCHUNK)
            nc.sync.dma_start(xt[:, :, cs], xt_dr[:, :, cs])
            if use_rope:
                nc.sync.dma_start(cos8[:, 4*c:4*c+4, :], cos_dr[:, 4*c:4*c+4, :])
                nc.sync.dma_start(sin8[:, 4*c:4*c+4, :], sin_dr[:, 4*c:4*c+4, :])
        nc.sync.dma_start(wo[:], wo_d.rearrange("(k p) o -> p k o", p=128))

        # ---------- emission helpers ----------
        def proj_mtile(m):
            """QKV projection + rope + transpose + V staging for s-tile m."""
            ms = slice(m * 128, (m + 1) * 128)
            ps = pp.tile([128, 2 * GDIM], F32,
                         tag=("ps_qk", "ps_v")[m % 2], name="ps")
            for k in range(KD):
                nc.tensor.matmul(ps[:], xt[:, k, ms], wqk[:, k, :],
                                 start=(k == 0), stop=(k == KD - 1))
            qkr = work.tile([128, 2 * GDIM], BF16, tag="qkr", name="qkr")
            if use_rope:
                qkf = work.tile([128, 2 * GDIM], BF16, tag="qkf", name="qkf")
                if m < 4:
                    nc.scalar.copy(qkf[:], ps[:])
                else:
                    nc.vector.tensor_copy(qkf[:], ps[:])
                # head dims are [evens(32) | odds(32)] per 64-block (host
                # permuted): E/O are 8 contiguous 32-col blocks at stride 64
                qv = qkf.rearrange("p (hb eo f) -> p hb eo f", eo=2, f=32)
                ov = qkr.rearrange("p (hb eo f) -> p hb eo f", eo=2, f=32)
                E, O = qv[:, :, 0, :], qv[:, :, 1, :]
                C = cos8[:, m, :].rearrange("p (hb f) -> p hb f", f=32)
                Sn = sin8[:, m, :].rearrange("p (hb f) -> p hb f", f=32)
                t_c = work.tile([128, 512], BF16, tag="tc", name="tc")
                t_s = work.tile([128, 512], BF16, tag="ts", name="ts")
                tcv = t_c.rearrange("p (hb eo f) -> p hb eo f", eo=2, f=32)
                tsv = t_s.rearrange("p (hb eo f) -> p hb eo f", eo=2, f=32)
                nc.vector.tensor_mul(tcv[:, :, 0, :], E, C)
                nc.vector.tensor_mul(tcv[:, :, 1, :], O, C)
                nc.vector.tensor_mul(tsv[:, :, 0, :], E, Sn)
                nc.vector.tensor_mul(tsv[:, :, 1, :], O, Sn)
                # e' = E*c - O*s ; o' = O*c + E*s
                nc.vector.tensor_sub(ov[:, :, 0, :], tcv[:, :, 0, :], tsv[:, :, 1, :])
                nc.vector.tensor_add(ov[:, :, 1, :], tcv[:, :, 1, :], tsv[:, :, 0, :])
            else:
                nc.vector.tensor_copy(qkr[:], ps[:])
            # one batched transpose: [128 s, 512 o] -> qkt4[:, 0:4, m-block]
            gms = slice(m * 128, (m + 1) * 128)
            nc.sync.dma_start_transpose(qkt4[:, :, gms], qkr[:])

            psv = yqp.tile([128, GDIM], F32, tag="yq", name="psv")
            for k in range(KD):
                nc.tensor.matmul(psv[:], xt[:, k, ms], wv[:, k, :],
                                 start=(k == 0), stop=(k == KD - 1))
            dst = vsb4[:, m, :, 0:64]
            src = psv.rearrange("p (h c) -> p h c", h=4)
            if m < 4:
                nc.scalar.copy(dst, src)
            else:
                nc.vector.tensor_copy(dst, src)

        def attention_scores_hp(qc, hp, pe_all):
            """Scores + exp + causal mask for one head pair of q-chunk qc."""
            groups, base, ncols = _score_layout(qc)
            if True:
                qt = qkt4[:, hp, :]
                kt = qkt4[:, 2 + hp, :]
                for grp in groups:
                    gbase = grp[0][3]
                    gcols = grp[-1][3] + grp[-1][2] - gbase
                    scs = [scp.tile([128, WIDE], F32, tag=f"sc{i}",
                                    name=f"sc{i}") for i in range(2)]
                    for i in range(2):
                        rows = slice(i * 64, i * 64 + 64)
                        for (kb, qoff, n, colbase) in grp:
                            o = colbase - gbase
                            nc.tensor.matmul(
                                scs[i][:, o:o + n],
                                kt[rows, kb * 128:(kb + 1) * 128],
                                qt[rows, qoff:qoff + n],
                                start=True, stop=True)
                    for i in range(2):
                        pe = pe_all[hp][i]
                        nc.scalar.activation(pe[:, gbase:gbase + gcols],
                                             scs[i][:, :gcols], EXP, scale=0.125)
                        for (kb, qoff, n, colbase) in grp:
                            if kb >= 4 * qc:  # diagonal block: causal mask
                                nc.gpsimd.tensor_mul(
                                    pe[:, colbase:colbase + 128],
                                    pe[:, colbase:colbase + 128], maskT[:])

        def attention_pv(qc, pe_all, qls=(0, 1, 2, 3)):
            """Flipped PV per q-block: out [128 q, 4 heads, 65], then
            normalize via the accumulated denominator column + transpose."""
            _, base, _ = _score_layout(qc)
            for ql in qls:
                qb = 4 * qc + ql
                yq = yqp.tile([128, 4, 65], F32, tag="yq", name="yq")
                for h in range(4):
                    hp, i = divmod(h, 2)
                    pe = pe_all[hp][i]
                    for kb in range(qb + 1):
                        off = 128 * ql if kb < 4 * qc else 128 * (qb - kb)
                        col = base[kb] + off
                        nc.tensor.matmul(
                            yq[:, h, :], pe[:, col:col + 128],
                            vsb[:, kb, h * 65:(h + 1) * 65],
                            start=(kb == 0), stop=(kb == qb))
                yq_sb = work.tile([128, 4, 65], F32, tag="yqsb", name="yqsb")
                if qc == 0:
                    nc.scalar.copy(yq_sb[:], yq[:])
                else:
                    nc.vector.tensor_copy(yq_sb[:], yq[:])
                rc = work.tile([128, 4], F32, tag="rc", name="rc")
                nc.vector.reciprocal(rc[:], yq_sb[:, :, 64])
                y_sb = work.tile([128, 4, 64], BF16, tag="ysb", name="ysb")
                for h in range(4):
                    nc.gpsimd.tensor_scalar(y_sb[:, h, :], yq_sb[:, h, 0:64],
                                            rc[:, h:h + 1], None, MULT)
                nc.sync.dma_start_transpose(
                    yt2[:, :, qb * 128:(qb + 1) * 128], y_sb[:])

        def oproj_m(m, tags=("ps_qk", "ps_v"), evac=("dve", "dve"), out_q="sp",
                    split_out=False):
            # po reuses the phase-1 projection PSUM banks (tags rotate) --
            # avoids a pool boundary, which would order o_proj after every
            # phase-1 instruction.  After the last exp, the sc tags can join
            # the rotation for a deeper po pipeline.
            ms = slice(m * 128, (m + 1) * 128)
            so = work.tile([128, D], BF16, tag="so", name="so", bufs=4)
            for nb in range(2):
                if tags[nb] in ("ps_qk", "ps_v"):
                    po = pp.tile([128, 512], F32, tag=tags[nb], name="po")
                elif tags[nb] == "yq":
                    po = yqp.tile([128, 512], F32, tag="yq", name="po")
                else:
                    po = scp.tile([128, WIDE], F32, tag=tags[nb], name="po")
                for k in range(2):
                    nc.tensor.matmul(po[:, 0:512], yt2[:, k, ms],
                                     wo[:, k, nb * 512:(nb + 1) * 512],
                                     start=(k == 0), stop=(k == 1))
                dst = so[:, nb * 512:(nb + 1) * 512]
                if evac[nb] == "dve":
                    nc.vector.tensor_copy(dst, po[:, 0:512])
                else:
                    nc.scalar.copy(dst, po[:, 0:512])
                if split_out:
                    eng = nc.scalar if (m + nb) % 2 == 0 else nc.sync
                    eng.dma_start(out_d[ms, nb * 512:(nb + 1) * 512], dst)
            if not split_out:
                if out_q == "sp":
                    nc.sync.dma_start(out_d[ms, :], so[:])
                else:
                    nc.scalar.dma_start(out_d[ms, :], so[:])

        # ---------- interleaved emission ----------
        # Fine-grained round-robin: each score-group's exp (ACT) is shadowed
        # by a projection m-tile (PE) so the PE stream never blocks on the
        # single-buffered score PSUM tiles.
        pp = es.enter_context(tc.tile_pool(name="pp", bufs=1, space="PSUM"))
        for m in range(0, 4):
            proj_mtile(m)
        attention_scores_hp(0, 0, pe_main)
        proj_mtile(4)
        attention_scores_hp(0, 1, pe_main)
        proj_mtile(5)
        proj_mtile(6)
        proj_mtile(7)
        attention_pv(0, pe_main)
        attention_scores_hp(1, 0, pe_main)
        proj_mtile(8)
        attention_scores_hp(1, 1, pe_main)
        proj_mtile(9)
        proj_mtile(10)
        proj_mtile(11)
        attention_pv(1, pe_main)
        attention_scores_hp(2, 0, pe_main)
        proj_mtile(12)
        attention_scores_hp(2, 1, pe_main)
        proj_mtile(13)
        proj_mtile(14)
        proj_mtile(15)
        # phase 1 done: free xt/w/cos/sin, carve qc3 probs buffers from the
        # freed region so exp(qc3) is independent of PV(qc2)
        ph1_ctx.__exit__(None, None, None)
        with tc.tile_pool(name="pe3p", bufs=1) as pe3p:
            pe3 = [[pe3p.tile([128, NCOLS], BF16, tag=f"pe3{hp}{i}",
                              name=f"pe3{hp}{i}") for i in range(2)]
                   for hp in range(2)]
            attention_scores_hp(3, 0, pe3)
            for m in range(0, 4):
                oproj_m(m)
            attention_pv(2, pe_main)
            attention_scores_hp(3, 1, pe3)
            for m in range(4, 8):
                oproj_m(m)
            oproj_m(8, out_q="act")
            oproj_m(9, out_q="sp")
            oproj_m(10, out_q="act")
            oproj_m(11, out_q="sp")
            # tail: all four PV chains first (their normalize->transpose
            # chains pipeline down DVE/Pool/SP while PE works), then the
            # last o_proj tiles with po rotating through 4 banks and out
            # DMAs alternating between the SP and ACT queues
            attention_pv(3, pe3)
            oproj_m(12, tags=("ps_qk", "ps_v"), evac=("dve", "act"), out_q="act")
            oproj_m(13, tags=("sc0", "sc1"), evac=("dve", "act"), out_q="sp")
            oproj_m(14, tags=("ps_qk", "ps_v"), evac=("dve", "act"), split_out=True)
            oproj_m(15, tags=("yq", "sc0"), evac=("dve", "act"), split_out=True)
        es.close()
    nc.compile()
    return nc


_PERM64 = np.concatenate([np.arange(0, 64, 2), np.arange(1, 64, 2)])


def _prep_core_inputs(x, Wq, Wk, Wv, Wo, cos_g, sin_g, use_rope):
    """Host-side shard + layout prep. Returns list of 8 input dicts."""
    maskT = np.tril(np.ones((128, 128), np.float32)).T.astype(_BF16)
    # 8 copies of the 32-wide tables: one per [evens|odds] head block
    cos8 = np.tile(cos_g, (1, 8)).astype(_BF16)
    sin8 = np.tile(sin_g, (1, 8)).astype(_BF16)
    maps = []
    for c in range(NCORES):
        b, g = divmod(c, HEADS_PER_CORE)
        rows = slice(g * GDIM, (g + 1) * GDIM)
        wq_g = Wq[rows]
        wk_g = Wk[rows]
        if use_rope:
            # per-head row permutation to [evens(32) | odds(32)] so device
            # rope works on contiguous blocks; scores invariant (q,k share it)
            wq_g = wq_g.reshape(HEADS_PER_CORE, HD, D)[:, _PERM64, :].reshape(GDIM, D)
            wk_g = wk_g.reshape(HEADS_PER_CORE, HD, D)[:, _PERM64, :].reshape(GDIM, D)
        wqk = np.concatenate([wq_g, wk_g], axis=0).T  # [D, 512]
        m = {
            "xt": np.ascontiguousarray(x[b].T).astype(_BF16),
            "wqk": np.ascontiguousarray(wqk).astype(_BF16),
            "wv": np.ascontiguousarray(Wv[rows].T).astype(_BF16),
            "wo": np.ascontiguousarray(Wo[:, rows].T).astype(_BF16),
            "maskT": maskT,
        }
        if use_rope:
            m["cos8"] = cos8
            m["sin8"] = sin8
        maps.append(m)
    return maps


def kernel(x, token_positions, use_rope, Wq, Wk, Wv, Wo, cos, sin):
    from concourse.bass_utils import run_bass_kernel_spmd

    x = np.asarray(x, np.float32)
    token_positions = np.asarray(token_positions)
    Wq = np.asarray(Wq, np.float32)
    Wk = np.asarray(Wk, np.float32)
    Wv = np.asarray(Wv, np.float32)
    Wo = np.asarray(Wo, np.float32)
    cos = np.asarray(cos, np.float32)
    sin = np.asarray(sin, np.float32)
    rope = bool(int(use_rope))

    cos_g = cos[token_positions]  # [S, 32]
    sin_g = sin[token_positions]

    if rope not in _cache:
        _cache[rope] = _build(rope)
    nc = _cache[rope]

    in_maps = _prep_core_inputs(x, Wq, Wk, Wv, Wo, cos_g, sin_g, rope)
    res = run_bass_kernel_spmd(nc, in_maps, list(range(NCORES)))

    out = np.zeros((B, S, D), np.float32)
    for c in range(NCORES):
        out[c // HEADS_PER_CORE] += res.results[c]["out"].astype(np.float32)
    return out


# revision 58
# speedup vs baseline: 1.0123x; 1.0123x over previous
"""Causal multi-head attention with RoPE for Trainium2, 8-core SPMD.

Problem: B=2, S=2048, D_MODEL=1024, H=16, HD=64, causal softmax(QK^T/8)V
with interleaved-pair RoPE on q/k, projections Wq/Wk/Wv/Wo.

Sharding (host side): batch x head-group. Core c handles batch b=c//4 and
head group g=c%4 (heads 4g..4g+3, a 256-wide slice of the projection dims).
Each core computes a full [S, D_MODEL] partial of the output (its head
group's contribution through Wo) in bf16; host casts to f32 and sums 4
partials per batch.

Device strategy (all matmuls bf16, fp32 accumulate):
 - emission interleaves projection m-tiles, attention q-chunks and o_proj
   tiles at score-group granularity so the PE stream always has work while
   ACT chews through the exp backlog; input DMAs are chunked so the first
   projection starts ~4.5us in
 - host permutes Wq/Wk rows per head to [evens(32) | odds(32)] so RoPE
   reads contiguous blocks: two full-width muls (qkf*cos16, qkf*sin16) +
   strided-block add/sub on DVE; scores are invariant to the permutation
   since q and k share it
 - Q,K projected in [s, o] layout -> RoPE -> one batched DMA transpose per
   m-tile into qkt4 [128, 4, S]; QK projection PSUM double-buffered across
   two tags, V accumulates via the PV psum ring
 - scoresT[k, q] = Kt.T @ Qt per 128-key block, head pairs row-packed on PE
   partitions 0:64/64:128; wide [128, 1024] PSUM score tiles, one Exp per
   (group, head) on ACT writing probs into per-(hp,head) SBUF buffers
   (qc0-2 share one set; qc3 gets its own carved from the released
   phase-1 pool so exp(qc3) never waits on PV(qc2)); causal diagonal
   masked by Pool multiply
 - PV flipped: out[q, h, hd] with lhsT = probs block [keys, q], rhs =
   [V | 1] [keys, 65] -- N=65 per matmul (the cost driver is the moving
   dim) instead of 128-512; col 64 accumulates the softmax denominator per
   q partition, so normalization is one reciprocal + per-partition
   tensor_scalar on Pool
 - y [q, hd] normalized then batch-transposed into yt2 [128, 2, S];
   o_proj per q-chunk with po PSUM rotating over freed phase-1/score
   banks, evacuation split DVE/ACT at the tail, out DMAs in bf16
   alternating between the SP and ACT DMA queues
"""

import numpy as np
import ml_dtypes

B, S, D, H = 2, 2048, 1024, 16
HD = 64
NCORES = 8
HEADS_PER_CORE = 4
GDIM = HEADS_PER_CORE * HD          # 256 projection cols per core
SB = S // 128                        # 16 s-tiles
KD = D // 128                        # 8 k-tiles over d
QCHUNK = 512
NQC = S // QCHUNK                    # 4 q-chunks
WIDE = 1024                          # wide scores psum tile (2 banks)

_BF16 = ml_dtypes.bfloat16
_cache = {}


def _score_layout(qc):
    """Per (qc): list of (kb, qoff, n) in emission order and the global column
    base of each kb block in the pe probs buffer; plus chunking into <=WIDE
    score-psum groups. Returns (groups, base) where groups is a list of
    [(kb, qoff, n, colbase), ...] and base maps kb -> global pe column."""
    q0 = qc * QCHUNK
    order = list(range(4 * qc)) + [4 * qc, 4 * qc + 1, 4 * qc + 3, 4 * qc + 2]
    base = {}
    blocks = []
    pos = 0
    for kb in order:
        r = max(0, kb - 4 * qc)
        qoff = q0 + r * 128 if kb >= 4 * qc else q0
        n = QCHUNK - r * 128 if kb >= 4 * qc else QCHUNK
        base[kb] = pos
        blocks.append((kb, qoff, n, pos))
        pos += n
    groups, cur, cols = [], [], 0
    for (kb, qoff, n, colbase) in blocks:
        if cols + n > WIDE:
            groups.append(cur)
            cur, cols = [], 0
        cur.append((kb, qoff, n, colbase))
        cols += n
    groups.append(cur)
    return groups, base, pos


def _build(use_rope: bool):
    import concourse.bass as bass
    import concourse.mybir as mybir
    import concourse.tile as tile
    from concourse import bacc
    from contextlib import ExitStack

    F32 = mybir.dt.float32
    BF16 = mybir.dt.bfloat16
    EXP = mybir.ActivationFunctionType.Exp
    MULT = mybir.AluOpType.mult

    nc = bacc.Bacc(None, target_bir_lowering=False)

    xt_d = nc.dram_tensor("xt", [D, S], BF16, kind="ExternalInput")
    wqk_d = nc.dram_tensor("wqk", [D, 2 * GDIM], BF16, kind="ExternalInput")
    wv_d = nc.dram_tensor("wv", [D, GDIM], BF16, kind="ExternalInput")
    wo_d = nc.dram_tensor("wo", [GDIM, D], BF16, kind="ExternalInput")
    cos_d = nc.dram_tensor("cos8", [S, 256], BF16, kind="ExternalInput")
    sin_d = nc.dram_tensor("sin8", [S, 256], BF16, kind="ExternalInput")
    mask_d = nc.dram_tensor("maskT", [128, 128], BF16, kind="ExternalInput")
    out_d = nc.dram_tensor("out", [S, D], BF16, kind="ExternalOutput")

    # pe probs buffer column count for the widest chunk (qc=3)
    _, _, NCOLS = _score_layout(NQC - 1)

    xt_dr = xt_d.rearrange("(k p) s -> p k s", p=128)
    wqk_dr = wqk_d.rearrange("(k p) o -> p k o", p=128)
    cos_dr = cos_d.rearrange("(m p) f -> p m f", p=128)
    sin_dr = sin_d.rearrange("(m p) f -> p m f", p=128)

    with tile.TileContext(nc) as tc:
        es = ExitStack()
        big = es.enter_context(tc.tile_pool(name="big", bufs=1))
        work = es.enter_context(tc.tile_pool(name="work", bufs=2))
        scp = es.enter_context(tc.tile_pool(name="sc", bufs=1, space="PSUM"))
        yqp = es.enter_context(tc.tile_pool(name="yq", bufs=2, space="PSUM"))

        # ---- resident tiles ----
        wo = big.tile([128, 2, D], BF16)
        maskT = big.tile([128, 128], BF16)
        qkt4 = big.tile([128, 4, S], BF16)
        vsb = big.tile([128, SB, HEADS_PER_CORE * 65], BF16)
        yt2 = big.tile([128, 2, S], BF16)
        # probs buffers for qc0-2 (max 5376 cols); qc3 gets its own buffers
        # carved out of the released phase-1 pool so exp(qc3) need not wait
        # for PV(qc2) to drain these
        _, _, NC2 = _score_layout(2)
        _, _, NC1 = _score_layout(1)
        pe_main = [[big.tile([128, NC2], BF16, tag=f"pe{hp}{i}",
                             name=f"pe{hp}{i}") for i in range(2)]
                   for hp in range(2)]
        # phase-1-only tensors: released after the last projection m-tile
        ph1_ctx = tc.tile_pool(name="ph1", bufs=1)
        ph1 = ph1_ctx.__enter__()
        xt = ph1.tile([128, KD, S], BF16)
        wqk = ph1.tile([128, KD, 2 * GDIM], BF16)
        wv = ph1.tile([128, KD, GDIM], BF16)
        if use_rope:
            cos8 = ph1.tile([128, SB, 256], BF16)
            sin8 = ph1.tile([128, SB, 256], BF16)

        # ones columns of [V | 1] (memset before anything else)
        vsb4 = vsb.rearrange("p m (h c) -> p m h c", h=4)
        nc.vector.memset(vsb4[:, :, :, 64:65], 1.0)

        # ---- input DMAs, chunked so m-tile 0 unblocks early; weights go
        # down the ACT queue in parallel with xt on the SP queue ----
        nc.sync.dma_start(wqk[:, 0:2, :], wqk_dr[:, 0:2, :])
        nc.sync.dma_start(xt[:, 0:2, 0:QCHUNK], xt_dr[:, 0:2, 0:QCHUNK])
        nc.sync.dma_start(wqk[:, 2:4, :], wqk_dr[:, 2:4, :])
        nc.sync.dma_start(xt[:, 2:4, 0:QCHUNK], xt_dr[:, 2:4, 0:QCHUNK])
        nc.sync.dma_start(wqk[:, 4:6, :], wqk_dr[:, 4:6, :])
        nc.sync.dma_start(xt[:, 4:6, 0:QCHUNK], xt_dr[:, 4:6, 0:QCHUNK])
        nc.sync.dma_start(wqk[:, 6:8, :], wqk_dr[:, 6:8, :])
        nc.sync.dma_start(xt[:, 6:8, 0:QCHUNK], xt_dr[:, 6:8, 0:QCHUNK])
        nc.sync.dma_start(wv[:], wv_d.rearrange("(k p) o -> p k o", p=128))
        if use_rope:
            nc.sync.dma_start(cos8[:, 0:4, :], cos_dr[:, 0:4, :])
            nc.sync.dma_start(sin8[:, 0:4, :], sin_dr[:, 0:4, :])
        nc.sync.dma_start(maskT[:], mask_d[:])
        for c in range(1, 4):
            cs = slice(c * QCHUNK, (c + 1) * QCHUNK)
            nc.sync.dma_start(xt[:, :, cs], xt_dr[:, :, cs])
            if use_rope:
                nc.sync.dma_start(cos8[:, 4*c:4*c+4, :], cos_dr[:, 4*c:4*c+4, :])
                nc.sync.dma_start(sin8[:, 4*c:4*c+4, :], sin_dr[:, 4*c:4*c+4, :])
        nc.sync.dma_start(wo[:], wo_d.rearrange("(k p) o -> p k o", p=128))

        # ---------- emission helpers ----------
        def proj_mtile(m):
            """QKV projection + rope + transpose + V staging for s-tile m."""
            ms = slice(m * 128, (m + 1) * 128)
            ps = pp.tile([128, 2 * GDIM], F32,
                         tag=("ps_qk", "ps_v")[m % 2], name="ps")
            for k in range(KD):
                nc.tensor.matmul(ps[:], xt[:, k, ms], wqk[:, k, :],
                                 start=(k == 0), stop=(k == KD - 1))
            qkr = work.tile([128, 2 * GDIM], BF16, tag="qkr", name="qkr")
            if use_rope:
                qkf = work.tile([128, 2 * GDIM], BF16, tag="qkf", name="qkf")
                if m < 4:
                    nc.scalar.copy(qkf[:], ps[:])
                else:
                    nc.vector.tensor_copy(qkf[:], ps[:])
                # head dims are [evens(32) | odds(32)] per 64-block (host
                # permuted): E/O are 8 contiguous 32-col blocks at stride 64
                qv = qkf.rearrange("p (hb eo f) -> p hb eo f", eo=2, f=32)
                ov = qkr.rearrange("p (hb eo f) -> p hb eo f", eo=2, f=32)
                E, O = qv[:, :, 0, :], qv[:, :, 1, :]
                C = cos8[:, m, :].rearrange("p (hb f) -> p hb f", f=32)
                Sn = sin8[:, m, :].rearrange("p (hb f) -> p hb f", f=32)
                t_c = work.tile([128, 512], BF16, tag="tc", name="tc")
                t_s = work.tile([128, 512], BF16, tag="ts", name="ts")
                tcv = t_c.rearrange("p (hb eo f) -> p hb eo f", eo=2, f=32)
                tsv = t_s.rearrange("p (hb eo f) -> p hb eo f", eo=2, f=32)
                nc.vector.tensor_mul(tcv[:, :, 0, :], E, C)
                nc.vector.tensor_mul(tcv[:, :, 1, :], O, C)
                nc.vector.tensor_mul(tsv[:, :, 0, :], E, Sn)
                nc.vector.tensor_mul(tsv[:, :, 1, :], O, Sn)
                # e' = E*c - O*s ; o' = O*c + E*s
                nc.vector.tensor_sub(ov[:, :, 0, :], tcv[:, :, 0, :], tsv[:, :, 1, :])
                nc.vector.tensor_add(ov[:, :, 1, :], tcv[:, :, 1, :], tsv[:, :, 0, :])
            else:
                nc.vector.tensor_copy(qkr[:], ps[:])
            # one batched transpose: [128 s, 512 o] -> qkt4[:, 0:4, m-block]
            gms = slice(m * 128, (m + 1) * 128)
            nc.sync.dma_start_transpose(qkt4[:, :, gms], qkr[:])

            psv = yqp.tile([128, GDIM], F32, tag="yq", name="psv")
            for k in range(KD):
                nc.tensor.matmul(psv[:], xt[:, k, ms], wv[:, k, :],
                                 start=(k == 0), stop=(k == KD - 1))
            dst = vsb4[:, m, :, 0:64]
            src = psv.rearrange("p (h c) -> p h c", h=4)
            if m < 4:
                nc.scalar.copy(dst, src)
            else:
                nc.vector.tensor_copy(dst, src)

        def attention_scores_hp(qc, hp, pe_all):
            """Scores + exp + causal mask for one head pair of q-chunk qc."""
            groups, base, ncols = _score_layout(qc)
            if True:
                qt = qkt4[:, hp, :]
                kt = qkt4[:, 2 + hp, :]
                for grp in groups:
                    gbase = grp[0][3]
                    gcols = grp[-1][3] + grp[-1][2] - gbase
                    scs = [scp.tile([128, WIDE], F32, tag=f"sc{i}",
                                    name=f"sc{i}") for i in range(2)]
                    for i in range(2):
                        rows = slice(i * 64, i * 64 + 64)
                        for (kb, qoff, n, colbase) in grp:
                            o = colbase - gbase
                            nc.tensor.matmul(
                                scs[i][:, o:o + n],
                                kt[rows, kb * 128:(kb + 1) * 128],
                                qt[rows, qoff:qoff + n],
                                start=True, stop=True)
                    for i in range(2):
                        pe = pe_all[hp][i]
                        nc.scalar.activation(pe[:, gbase:gbase + gcols],
                                             scs[i][:, :gcols], EXP, scale=0.125)
                        for (kb, qoff, n, colbase) in grp:
                            if kb >= 4 * qc:  # diagonal block: causal mask
                                nc.gpsimd.tensor_mul(
                                    pe[:, colbase:colbase + 128],
                                    pe[:, colbase:colbase + 128], maskT[:])

        def attention_pv(qc, pe_all, qls=(0, 1, 2, 3)):
            """Flipped PV per q-block: out [128 q, 4 heads, 65], then
            normalize via the accumulated denominator column + transpose."""
            _, base, _ = _score_layout(qc)
            for ql in qls:
                qb = 4 * qc + ql
                yq = yqp.tile([128, 4, 65], F32, tag="yq", name="yq")
                for h in range(4):
                    hp, i = divmod(h, 2)
                    pe = pe_all[hp][i]
                    for kb in range(qb + 1):
                        off = 128 * ql if kb < 4 * qc else 128 * (qb - kb)
                        col = base[kb] + off
                        nc.tensor.matmul(
                            yq[:, h, :], pe[:, col:col + 128],
                            vsb[:, kb, h * 65:(h + 1) * 65],
                            start=(kb == 0), stop=(kb == qb))
                yq_sb = work.tile([128, 4, 65], F32, tag="yqsb", name="yqsb", bufs=3)
                if qc == 0:
                    nc.scalar.copy(yq_sb[:], yq[:])
                else:
                    nc.vector.tensor_copy(yq_sb[:], yq[:])
                rc = work.tile([128, 4], F32, tag="rc", name="rc")
                nc.vector.reciprocal(rc[:], yq_sb[:, :, 64])
                y_sb = work.tile([128, 4, 64], BF16, tag="ysb", name="ysb", bufs=3)
                for h in range(4):
                    nc.gpsimd.tensor_scalar(y_sb[:, h, :], yq_sb[:, h, 0:64],
                                            rc[:, h:h + 1], None, MULT)
                nc.sync.dma_start_transpose(
                    yt2[:, :, qb * 128:(qb + 1) * 128], y_sb[:])

        def oproj_m(m, tags=("ps_qk", "ps_v"), evac=("dve", "dve"), out_q="sp",
                    split_out=False):
            # po reuses the phase-1 projection PSUM banks (tags rotate) --
            # avoids a pool boundary, which would order o_proj after every
            # phase-1 instruction.  After the last exp, the sc tags can join
            # the rotation for a deeper po pipeline.
            ms = slice(m * 128, (m + 1) * 128)
            so = work.tile([128, D], BF16, tag="so", name="so", bufs=6)
            for nb in range(2):
                if tags[nb] in ("ps_qk", "ps_v"):
                    po = pp.tile([128, 512], F32, tag=tags[nb], name="po")
                elif tags[nb] == "yq":
                    po = yqp.tile([128, 512], F32, tag="yq", name="po")
                else:
                    po = scp.tile([128, WIDE], F32, tag=tags[nb], name="po")
                for k in range(2):
                    nc.tensor.matmul(po[:, 0:512], yt2[:, k, ms],
                                     wo[:, k, nb * 512:(nb + 1) * 512],
                                     start=(k == 0), stop=(k == 1))
                dst = so[:, nb * 512:(nb + 1) * 512]
                if evac[nb] == "dve":
                    nc.vector.tensor_copy(dst, po[:, 0:512])
                else:
                    nc.scalar.copy(dst, po[:, 0:512])
                if split_out:
                    eng = nc.scalar if (m + nb) % 2 == 0 else nc.sync
                    eng.dma_start(out_d[ms, nb * 512:(nb + 1) * 512], dst)
            if not split_out:
                if out_q == "sp":
                    nc.sync.dma_start(out_d[ms, :], so[:])
                else:
                    nc.scalar.dma_start(out_d[ms, :], so[:])

        # ---------- interleaved emission ----------
        # Fine-grained round-robin: each score-group's exp (ACT) is shadowed
        # by a projection m-tile (PE) so the PE stream never blocks on the
        # single-buffered score PSUM tiles.
        pp = es.enter_context(tc.tile_pool(name="pp", bufs=1, space="PSUM"))
        for m in range(0, 4):
            proj_mtile(m)
        attention_scores_hp(0, 0, pe_main)
        proj_mtile(4)
        attention_scores_hp(0, 1, pe_main)
        proj_mtile(5)
        proj_mtile(6)
        proj_mtile(7)
        attention_pv(0, pe_main)
        attention_scores_hp(1, 0, pe_main)
        proj_mtile(8)
        attention_scores_hp(1, 1, pe_main)
        proj_mtile(9)
        proj_mtile(10)
        proj_mtile(11)
        attention_pv(1, pe_main)
        attention_scores_hp(2, 0, pe_main)
        proj_mtile(12)
        attention_scores_hp(2, 1, pe_main)
        proj_mtile(13)
        proj_mtile(14)
        proj_mtile(15)
        # phase 1 done: free xt/w/cos/sin, carve qc3 probs buffers from the
        # freed region so exp(qc3) is independent of PV(qc2)
        ph1_ctx.__exit__(None, None, None)
        with tc.tile_pool(name="pe3p", bufs=1) as pe3p:
            pe3 = [[pe3p.tile([128, NCOLS], BF16, tag=f"pe3{hp}{i}",
                              name=f"pe3{hp}{i}") for i in range(2)]
                   for hp in range(2)]
            attention_scores_hp(3, 0, pe3)
            for m in range(0, 4):
                oproj_m(m)
            attention_pv(2, pe_main)
            attention_scores_hp(3, 1, pe3)
            for m in range(4, 8):
                oproj_m(m)
            oproj_m(8, out_q="act")
            oproj_m(9, out_q="sp")
            oproj_m(10, out_q="act")
            oproj_m(11, out_q="sp")
            # tail: all four PV chains first (their normalize->transpose
            # chains pipeline down DVE/Pool/SP while PE works), then the
            # last o_proj tiles with po rotating through 4 banks and out
            # DMAs alternating between the SP and ACT queues
            attention_pv(3, pe3)
            oproj_m(12, tags=("ps_qk", "ps_v"), evac=("dve", "act"), out_q="act")
            oproj_m(13, tags=("sc0", "sc1"), evac=("dve", "act"), out_q="sp")
            oproj_m(14, tags=("ps_qk", "ps_v"), evac=("dve", "act"), split_out=True)
            oproj_m(15, tags=("yq", "sc0"), evac=("dve", "act"), split_out=True)
        es.close()
    nc.compile()
    return nc


_PERM64 = np.concatenate([np.arange(0, 64, 2), np.arange(1, 64, 2)])


def _prep_core_inputs(x, Wq, Wk, Wv, Wo, cos_g, sin_g, use_rope):
    """Host-side shard + layout prep. Returns list of 8 input dicts."""
    maskT = np.tril(np.ones((128, 128), np.float32)).T.astype(_BF16)
    # 8 copies of the 32-wide tables: one per [evens|odds] head block
    cos8 = np.tile(cos_g, (1, 8)).astype(_BF16)
    sin8 = np.tile(sin_g, (1, 8)).astype(_BF16)
    maps = []
    for c in range(NCORES):
        b, g = divmod(c, HEADS_PER_CORE)
        rows = slice(g * GDIM, (g + 1) * GDIM)
        wq_g = Wq[rows]
        wk_g = Wk[rows]
        if use_rope:
            # per-head row permutation to [evens(32) | odds(32)] so device
            # rope works on contiguous blocks; scores invariant (q,k share it)
            wq_g = wq_g.reshape(HEADS_PER_CORE, HD, D)[:, _PERM64, :].reshape(GDIM, D)
            wk_g = wk_g.reshape(HEADS_PER_CORE, HD, D)[:, _PERM64, :].reshape(GDIM, D)
        wqk = np.concatenate([wq_g, wk_g], axis=0).T  # [D, 512]
        m = {
            "xt": np.ascontiguousarray(x[b].T).astype(_BF16),
            "wqk": np.ascontiguousarray(wqk).astype(_BF16),
            "wv": np.ascontiguousarray(Wv[rows].T).astype(_BF16),
            "wo": np.ascontiguousarray(Wo[:, rows].T).astype(_BF16),
            "maskT": maskT,
        }
        if use_rope:
            m["cos8"] = cos8
            m["sin8"] = sin8
        maps.append(m)
    return maps


def kernel(x, token_positions, use_rope, Wq, Wk, Wv, Wo, cos, sin):
    from concourse.bass_utils import run_bass_kernel_spmd

    x = np.asarray(x, np.float32)
    token_positions = np.asarray(token_positions)
    Wq = np.asarray(Wq, np.float32)
    Wk = np.asarray(Wk, np.float32)
    Wv = np.asarray(Wv, np.float32)
    Wo = np.asarray(Wo, np.float32)
    cos = np.asarray(cos, np.float32)
    sin = np.asarray(sin, np.float32)
    rope = bool(int(use_rope))

    cos_g = cos[token_positions]  # [S, 32]
    sin_g = sin[token_positions]

    if rope not in _cache:
        _cache[rope] = _build(rope)
    nc = _cache[rope]

    in_maps = _prep_core_inputs(x, Wq, Wk, Wv, Wo, cos_g, sin_g, rope)
    res = run_bass_kernel_spmd(nc, in_maps, list(range(NCORES)))

    out = np.zeros((B, S, D), np.float32)
    for c in range(NCORES):
        out[c // HEADS_PER_CORE] += res.results[c]["out"].astype(np.float32)
    return out


# revision 62
# speedup vs baseline: 1.0155x; 1.0031x over previous
"""Causal multi-head attention with RoPE for Trainium2, 8-core SPMD.

Problem: B=2, S=2048, D_MODEL=1024, H=16, HD=64, causal softmax(QK^T/8)V
with interleaved-pair RoPE on q/k, projections Wq/Wk/Wv/Wo.

Sharding (host side): batch x head-group. Core c handles batch b=c//4 and
head group g=c%4 (heads 4g..4g+3, a 256-wide slice of the projection dims).
Each core computes a full [S, D_MODEL] partial of the output (its head
group's contribution through Wo) in bf16; host casts to f32 and sums 4
partials per batch.

Device strategy (all matmuls bf16, fp32 accumulate):
 - emission interleaves projection m-tiles, attention q-chunks and o_proj
   tiles at score-group granularity so the PE stream always has work while
   ACT chews through the exp backlog; input DMAs are chunked so the first
   projection starts ~4.5us in
 - host permutes Wq/Wk rows per head to [evens(32) | odds(32)] so RoPE
   reads contiguous blocks: two full-width muls (qkf*cos16, qkf*sin16) +
   strided-block add/sub on DVE; scores are invariant to the permutation
   since q and k share it
 - Q,K projected in [s, o] layout -> RoPE -> one batched DMA transpose per
   m-tile into qkt4 [128, 4, S]; QK projection PSUM double-buffered across
   two tags, V accumulates via the PV psum ring
 - scoresT[k, q] = Kt.T @ Qt per 128-key block, head pairs row-packed on PE
   partitions 0:64/64:128; wide [128, 1024] PSUM score tiles, one Exp per
   (group, head) on ACT writing probs into per-(hp,head) SBUF buffers
   (qc0-2 share one set; qc3 gets its own carved from the released
   phase-1 pool so exp(qc3) never waits on PV(qc2)); causal diagonal
   masked by Pool multiply
 - PV flipped: out[q, h, hd] with lhsT = probs block [keys, q], rhs =
   [V | 1] [keys, 65] -- N=65 per matmul (the cost driver is the moving
   dim) instead of 128-512; col 64 accumulates the softmax denominator per
   q partition, so normalization is one reciprocal + per-partition
   tensor_scalar on Pool
 - y [q, hd] normalized then batch-transposed into yt2 [128, 2, S];
   o_proj per q-chunk with po PSUM rotating over freed phase-1/score
   banks, evacuation split DVE/ACT at the tail, out DMAs in bf16
   alternating between the SP and ACT DMA queues
"""

import numpy as np
import ml_dtypes

B, S, D, H = 2, 2048, 1024, 16
HD = 64
NCORES = 8
HEADS_PER_CORE = 4
GDIM = HEADS_PER_CORE * HD          # 256 projection cols per core
SB = S // 128                        # 16 s-tiles
KD = D // 128                        # 8 k-tiles over d
QCHUNK = 512
NQC = S // QCHUNK                    # 4 q-chunks
WIDE = 1024                          # wide scores psum tile (2 banks)

_BF16 = ml_dtypes.bfloat16
_cache = {}


def _score_layout(qc):
    """Per (qc): list of (kb, qoff, n) in emission order and the global column
    base of each kb block in the pe probs buffer; plus chunking into <=WIDE
    score-psum groups. Returns (groups, base) where groups is a list of
    [(kb, qoff, n, colbase), ...] and base maps kb -> global pe column."""
    q0 = qc * QCHUNK
    order = list(range(4 * qc)) + [4 * qc, 4 * qc + 1, 4 * qc + 3, 4 * qc + 2]
    base = {}
    blocks = []
    pos = 0
    for kb in order:
        r = max(0, kb - 4 * qc)
        qoff = q0 + r * 128 if kb >= 4 * qc else q0
        n = QCHUNK - r * 128 if kb >= 4 * qc else QCHUNK
        base[kb] = pos
        blocks.append((kb, qoff, n, pos))
        pos += n
    groups, cur, cols = [], [], 0
    for (kb, qoff, n, colbase) in blocks:
        if cols + n > WIDE:
            groups.append(cur)
            cur, cols = [], 0
        cur.append((kb, qoff, n, colbase))
        cols += n
    groups.append(cur)
    return groups, base, pos


def _build(use_rope: bool):
    import concourse.bass as bass
    import concourse.mybir as mybir
    import concourse.tile as tile
    from concourse import bacc
    from contextlib import ExitStack

    F32 = mybir.dt.float32
    BF16 = mybir.dt.bfloat16
    EXP = mybir.ActivationFunctionType.Exp
    MULT = mybir.AluOpType.mult

    nc = bacc.Bacc(None, target_bir_lowering=False)

    xt_d = nc.dram_tensor("xt", [D, S], BF16, kind="ExternalInput")
    wqk_d = nc.dram_tensor("wqk", [D, 2 * GDIM], BF16, kind="ExternalInput")
    wv_d = nc.dram_tensor("wv", [D, GDIM], BF16, kind="ExternalInput")
    wo_d = nc.dram_tensor("wo", [GDIM, D], BF16, kind="ExternalInput")
    cos_d = nc.dram_tensor("cos8", [S, 256], BF16, kind="ExternalInput")
    sin_d = nc.dram_tensor("sin8", [S, 256], BF16, kind="ExternalInput")
    mask_d = nc.dram_tensor("maskT", [128, 128], BF16, kind="ExternalInput")
    out_d = nc.dram_tensor("out", [S, D], BF16, kind="ExternalOutput")

    # pe probs buffer column count for the widest chunk (qc=3)
    _, _, NCOLS = _score_layout(NQC - 1)

    xt_dr = xt_d.rearrange("(k p) s -> p k s", p=128)
    wqk_dr = wqk_d.rearrange("(k p) o -> p k o", p=128)
    cos_dr = cos_d.rearrange("(m p) f -> p m f", p=128)
    sin_dr = sin_d.rearrange("(m p) f -> p m f", p=128)

    with tile.TileContext(nc) as tc:
        es = ExitStack()
        big = es.enter_context(tc.tile_pool(name="big", bufs=1))
        work = es.enter_context(tc.tile_pool(name="work", bufs=2))
        scp = es.enter_context(tc.tile_pool(name="sc", bufs=1, space="PSUM"))
        yqp = es.enter_context(tc.tile_pool(name="yq", bufs=2, space="PSUM"))

        # ---- resident tiles ----
        wo = big.tile([128, 2, D], BF16)
        maskT = big.tile([128, 128], BF16)
        qkt4 = big.tile([128, 4, S], BF16)
        vsb = big.tile([128, SB, HEADS_PER_CORE * 65], BF16)
        yt2 = big.tile([128, 2, S], BF16)
        # probs buffers for qc0-2 (max 5376 cols); qc3 gets its own buffers
        # carved out of the released phase-1 pool so exp(qc3) need not wait
        # for PV(qc2) to drain these
        _, _, NC2 = _score_layout(2)
        _, _, NC1 = _score_layout(1)
        pe_main = [[big.tile([128, NC2], BF16, tag=f"pe{hp}{i}",
                             name=f"pe{hp}{i}") for i in range(2)]
                   for hp in range(2)]
        # phase-1-only tensors: released after the last projection m-tile
        ph1_ctx = tc.tile_pool(name="ph1", bufs=1)
        ph1 = ph1_ctx.__enter__()
        xt = ph1.tile([128, KD, S], BF16)
        wqk = ph1.tile([128, KD, 2 * GDIM], BF16)
        wv = ph1.tile([128, KD, GDIM], BF16)
        if use_rope:
            cos8 = ph1.tile([128, SB, 256], BF16)
            sin8 = ph1.tile([128, SB, 256], BF16)

        # ones columns of [V | 1] (memset before anything else)
        vsb4 = vsb.rearrange("p m (h c) -> p m h c", h=4)
        nc.vector.memset(vsb4[:, :, :, 64:65], 1.0)

        # ---- input DMAs, chunked so m-tile 0 unblocks early; weights go
        # down the ACT queue in parallel with xt on the SP queue ----
        nc.sync.dma_start(wqk[:, 0:2, :], wqk_dr[:, 0:2, :])
        nc.sync.dma_start(xt[:, 0:2, 0:QCHUNK], xt_dr[:, 0:2, 0:QCHUNK])
        nc.sync.dma_start(wqk[:, 2:4, :], wqk_dr[:, 2:4, :])
        nc.sync.dma_start(xt[:, 2:4, 0:QCHUNK], xt_dr[:, 2:4, 0:QCHUNK])
        nc.sync.dma_start(wqk[:, 4:6, :], wqk_dr[:, 4:6, :])
        nc.sync.dma_start(xt[:, 4:6, 0:QCHUNK], xt_dr[:, 4:6, 0:QCHUNK])
        nc.sync.dma_start(wqk[:, 6:8, :], wqk_dr[:, 6:8, :])
        nc.sync.dma_start(xt[:, 6:8, 0:QCHUNK], xt_dr[:, 6:8, 0:QCHUNK])
        nc.sync.dma_start(wv[:], wv_d.rearrange("(k p) o -> p k o", p=128))
        if use_rope:
            nc.sync.dma_start(cos8[:, 0:4, :], cos_dr[:, 0:4, :])
            nc.sync.dma_start(sin8[:, 0:4, :], sin_dr[:, 0:4, :])
        nc.sync.dma_start(maskT[:], mask_d[:])
        for c in range(1, 4):
            cs = slice(c * QCHUNK, (c + 1) * QCHUNK)
            nc.sync.dma_start(xt[:, :, cs], xt_dr[:, :, cs])
            if use_rope:
                nc.sync.dma_start(cos8[:, 4*c:4*c+4, :], cos_dr[:, 4*c:4*c+4, :])
                nc.sync.dma_start(sin8[:, 4*c:4*c+4, :], sin_dr[:, 4*c:4*c+4, :])
        nc.sync.dma_start(wo[:], wo_d.rearrange("(k p) o -> p k o", p=128))

        # ---------- emission helpers ----------
        def proj_mtile(m):
            """QKV projection + rope + transpose + V staging for s-tile m."""
            ms = slice(m * 128, (m + 1) * 128)
            ps = pp.tile([128, 2 * GDIM], F32,
                         tag=("ps_qk", "ps_v")[m % 2], name="ps")
            for k in range(KD):
                nc.tensor.matmul(ps[:], xt[:, k, ms], wqk[:, k, :],
                                 start=(k == 0), stop=(k == KD - 1))
            qkr = work.tile([128, 2 * GDIM], BF16, tag="qkr", name="qkr")
            if use_rope:
                qkf = work.tile([128, 2 * GDIM], BF16, tag="qkf", name="qkf")
                if m < 4:
                    nc.scalar.copy(qkf[:], ps[:])
                else:
                    nc.vector.tensor_copy(qkf[:], ps[:])
                # head dims are [evens(32) | odds(32)] per 64-block (host
                # permuted): E/O are 8 contiguous 32-col blocks at stride 64
                qv = qkf.rearrange("p (hb eo f) -> p hb eo f", eo=2, f=32)
                ov = qkr.rearrange("p (hb eo f) -> p hb eo f", eo=2, f=32)
                E, O = qv[:, :, 0, :], qv[:, :, 1, :]
                C = cos8[:, m, :].rearrange("p (hb f) -> p hb f", f=32)
                Sn = sin8[:, m, :].rearrange("p (hb f) -> p hb f", f=32)
                t_c = work.tile([128, 512], BF16, tag="tc", name="tc")
                t_s = work.tile([128, 512], BF16, tag="ts", name="ts")
                tcv = t_c.rearrange("p (hb eo f) -> p hb eo f", eo=2, f=32)
                tsv = t_s.rearrange("p (hb eo f) -> p hb eo f", eo=2, f=32)
                nc.vector.tensor_mul(tcv[:, :, 0, :], E, C)
                nc.vector.tensor_mul(tcv[:, :, 1, :], O, C)
                nc.vector.tensor_mul(tsv[:, :, 0, :], E, Sn)
                nc.vector.tensor_mul(tsv[:, :, 1, :], O, Sn)
                # e' = E*c - O*s ; o' = O*c + E*s
                nc.vector.tensor_sub(ov[:, :, 0, :], tcv[:, :, 0, :], tsv[:, :, 1, :])
                nc.vector.tensor_add(ov[:, :, 1, :], tcv[:, :, 1, :], tsv[:, :, 0, :])
            else:
                nc.vector.tensor_copy(qkr[:], ps[:])
            # one batched transpose: [128 s, 512 o] -> qkt4[:, 0:4, m-block]
            gms = slice(m * 128, (m + 1) * 128)
            nc.sync.dma_start_transpose(qkt4[:, :, gms], qkr[:])

            psv = yqp.tile([128, GDIM], F32, tag="yq", name="psv")
            for k in range(KD):
                nc.tensor.matmul(psv[:], xt[:, k, ms], wv[:, k, :],
                                 start=(k == 0), stop=(k == KD - 1))
            dst = vsb4[:, m, :, 0:64]
            src = psv.rearrange("p (h c) -> p h c", h=4)
            if m < 4:
                nc.scalar.copy(dst, src)
            else:
                nc.vector.tensor_copy(dst, src)

        def attention_scores_hp(qc, hp, pe_all):
            """Scores + exp + causal mask for one head pair of q-chunk qc."""
            groups, base, ncols = _score_layout(qc)
            if True:
                qt = qkt4[:, hp, :]
                kt = qkt4[:, 2 + hp, :]
                for grp in groups:
                    gbase = grp[0][3]
                    gcols = grp[-1][3] + grp[-1][2] - gbase
                    scs = [scp.tile([128, WIDE], F32, tag=f"sc{i}",
                                    name=f"sc{i}") for i in range(2)]
                    for i in range(2):
                        rows = slice(i * 64, i * 64 + 64)
                        for (kb, qoff, n, colbase) in grp:
                            o = colbase - gbase
                            nc.tensor.matmul(
                                scs[i][:, o:o + n],
                                kt[rows, kb * 128:(kb + 1) * 128],
                                qt[rows, qoff:qoff + n],
                                start=True, stop=True)
                    for i in range(2):
                        pe = pe_all[hp][i]
                        nc.scalar.activation(pe[:, gbase:gbase + gcols],
                                             scs[i][:, :gcols], EXP, scale=0.125)
                        for (kb, qoff, n, colbase) in grp:
                            if kb >= 4 * qc:  # diagonal block: causal mask
                                # DVE is busiest in the phase-1 window, so
                                # early chunks mask on Pool, late on DVE
                                eng = nc.gpsimd if qc < 2 else nc.vector
                                eng.tensor_mul(
                                    pe[:, colbase:colbase + 128],
                                    pe[:, colbase:colbase + 128], maskT[:])

        def attention_pv(qc, pe_all, qls=(0, 1, 2, 3)):
            """Flipped PV per q-block: out [128 q, 4 heads, 65], then
            normalize via the accumulated denominator column + transpose."""
            _, base, _ = _score_layout(qc)
            for ql in qls:
                qb = 4 * qc + ql
                yq = yqp.tile([128, 4, 65], F32, tag="yq", name="yq")
                for h in range(4):
                    hp, i = divmod(h, 2)
                    pe = pe_all[hp][i]
                    for kb in range(qb + 1):
                        off = 128 * ql if kb < 4 * qc else 128 * (qb - kb)
                        col = base[kb] + off
                        nc.tensor.matmul(
                            yq[:, h, :], pe[:, col:col + 128],
                            vsb[:, kb, h * 65:(h + 1) * 65],
                            start=(kb == 0), stop=(kb == qb))
                yq_sb = work.tile([128, 4, 65], F32, tag="yqsb", name="yqsb", bufs=3)
                if qc == 0:
                    nc.scalar.copy(yq_sb[:], yq[:])
                else:
                    nc.vector.tensor_copy(yq_sb[:], yq[:])
                rc = work.tile([128, 4], F32, tag="rc", name="rc")
                nc.vector.reciprocal(rc[:], yq_sb[:, :, 64])
                y_sb = work.tile([128, 4, 64], BF16, tag="ysb", name="ysb", bufs=3)
                for h in range(4):
                    nc.gpsimd.tensor_scalar(y_sb[:, h, :], yq_sb[:, h, 0:64],
                                            rc[:, h:h + 1], None, MULT)
                nc.sync.dma_start_transpose(
                    yt2[:, :, qb * 128:(qb + 1) * 128], y_sb[:])

        def oproj_m(m, tags=("ps_qk", "ps_v"), evac=("dve", "dve"), out_q="sp",
                    split_out=False):
            # po reuses the phase-1 projection PSUM banks (tags rotate) --
            # avoids a pool boundary, which would order o_proj after every
            # phase-1 instruction.  After the last exp, the sc tags can join
            # the rotation for a deeper po pipeline.
            ms = slice(m * 128, (m + 1) * 128)
            so = work.tile([128, D], BF16, tag="so", name="so", bufs=6)
            for nb in range(2):
                if tags[nb] in ("ps_qk", "ps_v"):
                    po = pp.tile([128, 512], F32, tag=tags[nb], name="po")
                elif tags[nb] == "yq":
                    po = yqp.tile([128, 512], F32, tag="yq", name="po")
                else:
                    po = scp.tile([128, WIDE], F32, tag=tags[nb], name="po")
                for k in range(2):
                    nc.tensor.matmul(po[:, 0:512], yt2[:, k, ms],
                                     wo[:, k, nb * 512:(nb + 1) * 512],
                                     start=(k == 0), stop=(k == 1))
                dst = so[:, nb * 512:(nb + 1) * 512]
                if evac[nb] == "dve":
                    nc.vector.tensor_copy(dst, po[:, 0:512])
                else:
                    nc.scalar.copy(dst, po[:, 0:512])
                if split_out:
                    eng = nc.scalar if (m + nb) % 2 == 0 else nc.sync
                    eng.dma_start(out_d[ms, nb * 512:(nb + 1) * 512], dst)
            if not split_out:
                if out_q == "sp":
                    nc.sync.dma_start(out_d[ms, :], so[:])
                else:
                    nc.scalar.dma_start(out_d[ms, :], so[:])

        # ---------- interleaved emission ----------
        # Fine-grained round-robin: each score-group's exp (ACT) is shadowed
        # by a projection m-tile (PE) so the PE stream never blocks on the
        # single-buffered score PSUM tiles.
        pp = es.enter_context(tc.tile_pool(name="pp", bufs=1, space="PSUM"))
        for m in range(0, 4):
            proj_mtile(m)
        attention_scores_hp(0, 0, pe_main)
        proj_mtile(4)
        attention_scores_hp(0, 1, pe_main)
        proj_mtile(5)
        proj_mtile(6)
        proj_mtile(7)
        attention_pv(0, pe_main)
        attention_scores_hp(1, 0, pe_main)
        proj_mtile(8)
        attention_scores_hp(1, 1, pe_main)
        proj_mtile(9)
        proj_mtile(10)
        proj_mtile(11)
        attention_pv(1, pe_main)
        attention_scores_hp(2, 0, pe_main)
        proj_mtile(12)
        attention_scores_hp(2, 1, pe_main)
        proj_mtile(13)
        proj_mtile(14)
        proj_mtile(15)
        # phase 1 done: free xt/w/cos/sin, carve qc3 probs buffers from the
        # freed region so exp(qc3) is independent of PV(qc2)
        ph1_ctx.__exit__(None, None, None)
        with tc.tile_pool(name="pe3p", bufs=1) as pe3p:
            pe3 = [[pe3p.tile([128, NCOLS], BF16, tag=f"pe3{hp}{i}",
                              name=f"pe3{hp}{i}") for i in range(2)]
                   for hp in range(2)]
            attention_scores_hp(3, 0, pe3)
            for m in range(0, 4):
                oproj_m(m)
            attention_pv(2, pe_main)
            attention_scores_hp(3, 1, pe3)
            for m in range(4, 8):
                oproj_m(m)
            oproj_m(8, out_q="act")
            oproj_m(9, out_q="sp")
            oproj_m(10, out_q="act")
            oproj_m(11, out_q="sp")
            # tail: all four PV chains first (their normalize->transpose
            # chains pipeline down DVE/Pool/SP while PE works), then the
            # last o_proj tiles with po rotating through 4 banks and out
            # DMAs alternating between the SP and ACT queues
            attention_pv(3, pe3)
            oproj_m(12, tags=("ps_qk", "ps_v"), evac=("dve", "act"), out_q="act")
            oproj_m(13, tags=("sc0", "sc1"), evac=("dve", "act"), out_q="sp")
            oproj_m(14, tags=("ps_qk", "ps_v"), evac=("dve", "act"), split_out=True)
            oproj_m(15, tags=("yq", "sc0"), evac=("dve", "act"), split_out=True)
        es.close()
    nc.compile()
    return nc


_PERM64 = np.concatenate([np.arange(0, 64, 2), np.arange(1, 64, 2)])


def _prep_core_inputs(x, Wq, Wk, Wv, Wo, cos_g, sin_g, use_rope):
    """Host-side shard + layout prep. Returns list of 8 input dicts."""
    maskT = np.tril(np.ones((128, 128), np.float32)).T.astype(_BF16)
    # 8 copies of the 32-wide tables: one per [evens|odds] head block
    cos8 = np.tile(cos_g, (1, 8)).astype(_BF16)
    sin8 = np.tile(sin_g, (1, 8)).astype(_BF16)
    maps = []
    for c in range(NCORES):
        b, g = divmod(c, HEADS_PER_CORE)
        rows = slice(g * GDIM, (g + 1) * GDIM)
        wq_g = Wq[rows]
        wk_g = Wk[rows]
        if use_rope:
            # per-head row permutation to [evens(32) | odds(32)] so device
            # rope works on contiguous blocks; scores invariant (q,k share it)
            wq_g = wq_g.reshape(HEADS_PER_CORE, HD, D)[:, _PERM64, :].reshape(GDIM, D)
            wk_g = wk_g.reshape(HEADS_PER_CORE, HD, D)[:, _PERM64, :].reshape(GDIM, D)
        wqk = np.concatenate([wq_g, wk_g], axis=0).T  # [D, 512]
        m = {
            "xt": np.ascontiguousarray(x[b].T).astype(_BF16),
            "wqk": np.ascontiguousarray(wqk).astype(_BF16),
            "wv": np.ascontiguousarray(Wv[rows].T).astype(_BF16),
            "wo": np.ascontiguousarray(Wo[:, rows].T).astype(_BF16),
            "maskT": maskT,
        }
        if use_rope:
            m["cos8"] = cos8
            m["sin8"] = sin8
        maps.append(m)
    return maps


def kernel(x, token_positions, use_rope, Wq, Wk, Wv, Wo, cos, sin):
    from concourse.bass_utils import run_bass_kernel_spmd

    x = np.asarray(x, np.float32)
    token_positions = np.asarray(token_positions)
    Wq = np.asarray(Wq, np.float32)
    Wk = np.asarray(Wk, np.float32)
    Wv = np.asarray(Wv, np.float32)
    Wo = np.asarray(Wo, np.float32)
    cos = np.asarray(cos, np.float32)
    sin = np.asarray(sin, np.float32)
    rope = bool(int(use_rope))

    cos_g = cos[token_positions]  # [S, 32]
    sin_g = sin[token_positions]

    if rope not in _cache:
        _cache[rope] = _build(rope)
    nc = _cache[rope]

    in_maps = _prep_core_inputs(x, Wq, Wk, Wv, Wo, cos_g, sin_g, rope)
    res = run_bass_kernel_spmd(nc, in_maps, list(range(NCORES)))

    out = np.zeros((B, S, D), np.float32)
    for c in range(NCORES):
        out[c // HEADS_PER_CORE] += res.results[c]["out"].astype(np.float32)
    return out
